# revision 29
# baseline (speedup 1.0000x reference)
"""Multi-head attention TRN2 kernel (nn_MultiHeadAttention_69922067579127).

Full-input contract: kernel(**inputs) takes the complete tensors and
returns the complete output. Sharding: batch x head-group hybrid —
core c = (batch c//2, head-group c%2) handles one batch (2048 tokens)
and 8 heads. The host pre-permutes all per-core operands (x^T, QKV/out
weight tiles in bf16) and sums the two per-group partial output
projections per batch, adding b_out once.

All matmuls run in bf16 with fp32 PSUM accumulation; rel err ~7e-3
against the 2e-2 gate.

The kernel is a two-engine balance: PE true work ~283us (QKV 82,
scores 55 concurrent-pair, AV 109, outproj 27, misc ~10) vs ACT exp
~285us (256 x [128,1024] EXP at 1114ns). The schedule keeps both
saturated:

  - scores: two K=64 head matmuls on row groups 0/64 run CONCURRENTLY
    on the PE (hw row-group tiling) — one 512-row slot per kt.
  - the 16 sweeps form ONE flat software-pipelined stream: scores run
    2 kt ahead of av, ACROSS sweep boundaries, so the ACT exp stream
    never sees a boundary bubble. The previous sweep's last two avs,
    its psum->SBUF drains, and its deferred normalization (broadcast
    matmul + reciprocal + multiplies + h1 partition-move DMA) all
    interleave into the next sweep's first kts.
  - per-kt PE slack (~475ns under the 1114ns ACT pace) is filled from
    a priority deque of filler generators (JIT-ordered next-pair QKV,
    vtok transposes, deferred norms, output projections), pumped by
    approximate PE-cost accounting. Pair-p QKV spills naturally into
    pair p's own first sweep (deadline: k chunk c by kt 4c).
  - prologue: DMA ordered so all of k0/q0/v0 weights + x chunk0 land
    ASAP on both queues; only k/q/v chunk0 + vtok0 of pair 0 run
    before the first sweep.

Emission-order safety: Tile derives dependencies from program order,
so a consumer emitted before its producer reads stale data silently.
The `ready` counters + ensure() force-pump the filler until a needed
k/q/vtok chunk's instructions are emitted before any scores/av that
reads them (this is what makes the JIT spillover of a pair's QKV into
its own first sweep safe).

Engine budget per core: PE ~306us true work (scores pair runs at
~320ns — row-tiled concurrency is ~1.5x, not 2x), ACT exp 287us, DVE
~160us. Measured HW exec ~390us on a cool device (vs 410us for the
previous schedule, 752us original baseline); hot-device (DVFS) runs
of identical code measure ~470us. Remaining gap to the ~330us floor:
~20us DMA/NEFF init prologue, ~16us pair-3/outproj tail, per-kt
ldweights/transition friction (~120ns/kt).
"""

import sys

sys.path.insert(0, "/opt/trn_rl_repo")

from collections import deque
from contextlib import ExitStack

import ml_dtypes
import numpy as np

import concourse.bacc as bacc
import concourse.mybir as mybir
import concourse.tile as tile
from concourse.bass_utils import run_bass_kernel_spmd
from concourse.masks import make_identity

F32 = mybir.dt.float32
BF16 = mybir.dt.bfloat16
EXP = mybir.ActivationFunctionType.Exp

B, T, D = 4, 2048, 1024
H, Dh = 16, 64
NCORES = 8
NPAIR = 4             # head-pairs per core (8 heads)
TC = 512              # token chunk for QKV
NTC = T // TC         # 4
KT = T // 128         # 16 key tiles
QC = 512              # queries per sweep
NSW = T // QC         # 4 sweeps

_CACHE = {}


class Filler:
    """Priority deque of filler generators yielding approx PE-ns costs."""

    def __init__(self):
        self.q = deque()
        self.debt = 0

    def push_front(self, gen):
        self.q.appendleft(gen)

    def push_back(self, gen):
        self.q.append(gen)

    def pump(self, ns):
        # debt-carrying: average consumption matches the budget even
        # though generator yield costs are coarse (a 426ns QKV yield
        # against a 450ns budget must not consume two yields).
        ns -= self.debt
        self.debt = 0
        while ns > 0 and self.q:
            try:
                c = next(self.q[0])
            except StopIteration:
                self.q.popleft()
                continue
            ns -= max(c if c else 0, 40)
        if ns < 0:
            self.debt = -ns

    def drain(self):
        while self.q:
            for _ in self.q.popleft():
                pass


def _build():
    nc = bacc.Bacc("TRN2", target_bir_lowering=False, debug=False)
    # host pre-transposed x^T: [ki, ko*T]
    x = nc.dram_tensor("x", [128, 8 * T], BF16, kind="ExternalInput").ap()
    # host pre-permuted: wqkv [ki, m*ko*n], bqkv [p, m], wout [p, m*n]
    wqkv = nc.dram_tensor("wqkv", [128, 12 * 8 * 128], BF16, kind="ExternalInput").ap()
    bqkv = nc.dram_tensor("bqkv", [128, 12], F32, kind="ExternalInput").ap()
    wout = nc.dram_tensor("wout", [128, 4 * D], BF16, kind="ExternalInput").ap()
    out = nc.dram_tensor("out", [T, D], BF16, kind="ExternalOutput").ap()

    with tile.TileContext(nc) as tc, ExitStack() as ctx:
        const = ctx.enter_context(tc.tile_pool(name="const", bufs=1))
        big = ctx.enter_context(tc.tile_pool(name="big", bufs=1))
        stp = ctx.enter_context(tc.tile_pool(name="stp", bufs=4))
        work = ctx.enter_context(tc.tile_pool(name="work", bufs=2))
        avsp = ctx.enter_context(tc.tile_pool(name="avsp", bufs=2))
        outp = ctx.enter_context(tc.tile_pool(name="outp", bufs=3))
        # PSUM: 8 banks. sc 2x2 + av 2x1 + mm 2x1 = 8.
        pssc = ctx.enter_context(tc.tile_pool(name="pssc", bufs=2, space="PSUM"))
        psav = ctx.enter_context(tc.tile_pool(name="psav", bufs=2, space="PSUM"))
        psA = ctx.enter_context(tc.tile_pool(name="psA", bufs=2, space="PSUM"))

        # ---- constants ----
        ones_b = const.tile([128, 64], BF16)
        nc.vector.memset(ones_b, 1.0)
        ident = const.tile([128, 128], BF16)
        make_identity(nc, ident)

        # ---- persistent per-core tiles ----
        xt = big.tile([128, 8, T], BF16)       # x^T
        qT = big.tile([128, NPAIR, T], BF16)   # per pair: [h_even|h_odd] dims
        kT = big.tile([128, NPAIR, T], BF16)
        vTt = big.tile([128, NPAIR, T], BF16)  # V^T staging (transpose source)
        # token-major V: per (kt, pair): [v_h0(64) | 1 1 | v_h1(64) | 1 1]
        vtok = big.tile([128, KT, NPAIR, 132], BF16)
        attnT = big.tile([128, NPAIR, T], BF16)

        vtok5 = vtok.rearrange("a k p (h c) -> a k p h c", c=66)
        nc.gpsimd.memset(vtok5[:, :, :, :, 64:66], 1.0)

        w_r = const.tile([128, 12, 8, 128], BF16)
        wq_v = wqkv.rearrange("a (m f) -> a m f", m=12)
        w_rv = w_r.rearrange("a m ko n -> a m (ko n)")
        bq_sb = const.tile([128, 12], F32)
        wo_r = const.tile([128, 4, D], BF16)
        x_v = x.rearrange("a (ko t) -> a ko t", ko=8)

        # ---- DMA prologue: JIT ordering, both queues balanced.
        # Queue BW ~135GB/s each; everything ordered by first-use
        # deadline (x chunk c by scores kt 4c, w m-tile by its QKV).
        def _x(q, tci, half):
            ko = slice(0, 4) if half == 0 else slice(4, 8)
            q.dma_start(
                out=xt[:, ko, tci * TC : (tci + 1) * TC],
                in_=x_v[:, ko, tci * TC : (tci + 1) * TC],
            )

        def _w(q, m):
            q.dma_start(out=w_rv[:, m : m + 1], in_=wq_v[:, m : m + 1])

        def _x2(q, tci, kolo, kohi):
            q.dma_start(
                out=xt[:, kolo:kohi, tci * TC : (tci + 1) * TC],
                in_=x_v[:, kolo:kohi, tci * TC : (tci + 1) * TC],
            )

        _x2(nc.sync, 0, 0, 2)
        _w(nc.scalar, 0)                     # k0 w
        nc.scalar.dma_start(out=bq_sb, in_=bqkv)
        _x2(nc.sync, 0, 2, 4)
        _x2(nc.scalar, 0, 4, 6)
        _x2(nc.scalar, 0, 6, 8)
        _w(nc.sync, 2)                       # q0 w
        _w(nc.scalar, 1)                     # v0 w
        _x(nc.sync, 1, 0)
        _x(nc.scalar, 1, 1)
        _x(nc.sync, 2, 0)
        _x(nc.scalar, 2, 1)
        _w(nc.sync, 3)                       # pair1 k w
        _w(nc.sync, 5)                       # pair1 q w
        _x(nc.sync, 3, 0)
        _x(nc.scalar, 3, 1)
        _w(nc.scalar, 4)                     # pair1 v w
        _w(nc.sync, 6)
        nc.sync.dma_start(out=wo_r, in_=wout.rearrange("a (m n) -> a m n", m=4))
        _w(nc.scalar, 8)
        _w(nc.scalar, 7)
        _w(nc.sync, 9)
        _w(nc.scalar, 11)
        _w(nc.sync, 10)

        # emission-order readiness: ready[(p, kind)] counts chunks whose
        # instructions are fully EMITTED; consumers must ensure() their
        # producer is emitted first or Tile sees read-before-write.
        ready = {}
        for _p in range(NPAIR):
            for _kind in ("k", "v", "q", "vt"):
                ready[(_p, _kind)] = 0

        def _qkv_chunk(p, mi, tci):
            """Generator: one QKV m-tile for one 512-token chunk."""
            m = 3 * p + mi
            t0 = tci * TC
            psq = psA.tile([128, TC], F32, tag="mm", name="psq")
            for ko in range(8):
                nc.tensor.matmul(
                    psq,
                    w_r[:, m, ko, :],
                    xt[:, ko, t0 : t0 + TC],
                    start=(ko == 0),
                    stop=(ko == 7),
                )
                if ko < 7:
                    yield 213
            dst = (kT, vTt, qT)[mi]
            nc.vector.tensor_scalar_add(
                out=dst[:, p, t0 : t0 + TC],
                in0=psq,
                scalar1=bq_sb[:, m : m + 1],
            )
            ready[(p, ("k", "v", "q")[mi])] += 1
            yield 213

        def _vtok_chunk(p, tci):
            """Generator: token-major V for one chunk's 4 key-tiles."""
            pst = psA.tile([128, TC], BF16, tag="mm", name="pst")
            for j in range(4):
                kt0 = tci * 4
                nc.tensor.transpose(
                    pst[:, j * 128 : (j + 1) * 128],
                    vTt[:, p, (kt0 + j) * 128 : (kt0 + j + 1) * 128],
                    ident,
                )
                yield 53
            nc.vector.tensor_copy(
                out=vtok5[:, tci * 4 : tci * 4 + 4, p, :, 0:64],
                in_=pst.rearrange("a (j h c) -> a j h c", j=4, c=64),
            )
            ready[(p, "vt")] += 1
            yield 40

        def head_jit(p, skip0=False):
            """Pair-p phase A in just-in-time order: k/q/v chunk 0 first,
            then per chunk c: k(c), v(c), vtok(c); q chunks 1-3 last
            (deadline: sweep sw needs q chunk sw)."""
            if not skip0:
                yield from _qkv_chunk(p, 0, 0)
                yield from _qkv_chunk(p, 2, 0)
            yield from _qkv_chunk(p, 1, 0)
            yield from _vtok_chunk(p, 0)
            for tci in range(1, NTC):
                yield from _qkv_chunk(p, 0, tci)
                yield from _qkv_chunk(p, 1, tci)
                yield from _vtok_chunk(p, tci)

        def phase_a_tail(p):
            """q chunks 1-3: chunk sw only needed by sweep sw."""
            for tci in range(1, NTC):
                yield from _qkv_chunk(p, 2, tci)

        def outproj(sw, tail=False):
            """Generator: output projection for sweep sw's 512 tokens,
            psum-accumulated over all 4 pairs."""
            for si in range(4):
                sl = sw * 4 + si
                outsb = outp.tile([128, D], BF16, tag="outsb", name="outsb")
                pos = [
                    psA.tile([128, QC], F32, tag="mm", name=f"po{n2}")
                    for n2 in range(2)
                ]
                for p in range(NPAIR):
                    for n2 in range(2):
                        nc.tensor.matmul(
                            pos[n2],
                            attnT[:, p, sl * 128 : (sl + 1) * 128],
                            wo_r[:, p, n2 * QC : (n2 + 1) * QC],
                            start=(p == 0),
                            stop=(p == NPAIR - 1),
                        )
                    yield 426
                nc.vector.tensor_copy(out=outsb[:, 0:QC], in_=pos[0])
                if tail:  # ACT is idle after the last exp
                    nc.scalar.activation(
                        out=outsb[:, QC : 2 * QC],
                        in_=pos[1],
                        func=mybir.ActivationFunctionType.Copy,
                    )
                else:  # during attention ACT paces the kt loop - use DVE
                    nc.vector.tensor_copy(
                        out=outsb[:, QC : 2 * QC], in_=pos[1]
                    )
                for n2 in range(2):
                    eng = nc.sync if (sl + n2) % 2 == 0 else nc.scalar
                    eng.dma_start(
                        out=out[sl * 128 : (sl + 1) * 128, n2 * QC : (n2 + 1) * QC],
                        in_=outsb[:, n2 * QC : (n2 + 1) * QC],
                    )
                yield 40

        def norm_gen(p, sw, avs0, avs1):
            """Deferred normalization of sweep (p, sw) from the SBUF
            drains: approx reciprocal on the denominator row -> DMA
            partition-broadcast (replaces the PE ones-matmul) ->
            multiply into attnT; h1 rows move to partitions 64:128."""
            q0 = sw * QC
            for h, avs in ((0, avs0), (1, avs1)):
                denrow = work.tile([1, QC], F32, tag=f"denrow{h}", name=f"denrow{h}")
                eng = nc.sync if h == 0 else nc.scalar
                eng.dma_start(out=denrow, in_=avs[64:65, :])
                yield 0
                recrow = work.tile([1, QC], F32, tag=f"recrow{h}", name=f"recrow{h}")
                nc.vector.reciprocal_approx_fast(out=recrow, in_=denrow)
                yield 40
                rec = work.tile([64, QC], F32, tag=f"rec{h}", name=f"rec{h}")
                nc.gpsimd.partition_broadcast(rec, recrow)
                yield 0
                if h == 0:
                    nc.vector.tensor_mul(
                        out=attnT[0:64, p, q0 : q0 + QC], in0=avs[0:64, :], in1=rec
                    )
                    yield 40
                else:
                    tmp1 = work.tile([64, QC], BF16, tag="tmp1", name="tmp1")
                    nc.vector.tensor_mul(out=tmp1, in0=avs[0:64, :], in1=rec)
                    yield 40
                    nc.sync.dma_start(
                        out=attnT[64:128, p, q0 : q0 + QC], in_=tmp1
                    )
                    yield 0

        class SweepState:
            """One 512-query sweep's score/av emission state."""

            def __init__(self, p, sw):
                self.p, self.sw = p, sw
                self.q0 = sw * QC
                self.sts = [None] * KT
                self.av0 = self.av1 = None

            def scores(self, kt):
                p, q0 = self.p, self.q0
                sc = pssc.tile([128, 1024], F32, tag="sc", name="sc")
                nc.tensor.matmul(
                    sc[:, 0:QC],
                    kT[0:64, p, kt * 128 : (kt + 1) * 128],
                    qT[0:64, p, q0 : q0 + QC],
                    start=True,
                    stop=True,
                )
                nc.tensor.matmul(
                    sc[:, QC : 2 * QC],
                    kT[64:128, p, kt * 128 : (kt + 1) * 128],
                    qT[64:128, p, q0 : q0 + QC],
                    start=True,
                    stop=True,
                )
                st = stp.tile([128, 1024], BF16, tag="st", name="st")
                nc.scalar.activation(out=st, in_=sc, func=EXP, scale=0.125)
                self.sts[kt] = st

            def av(self, kt):
                p = self.p
                if self.av0 is None:
                    self.av0 = psav.tile([66, QC], F32, tag="av", name="av0")
                    self.av1 = psav.tile([66, QC], F32, tag="av", name="av1")
                st = self.sts[kt]
                nc.tensor.matmul(
                    self.av0,
                    vtok[:, kt, p, 0:66],
                    st[:, 0:QC],
                    start=(kt == 0),
                    stop=(kt == KT - 1),
                )
                nc.tensor.matmul(
                    self.av1,
                    vtok[:, kt, p, 66:132],
                    st[:, QC : 2 * QC],
                    start=(kt == 0),
                    stop=(kt == KT - 1),
                )
                self.sts[kt] = None

            def drain(self):
                avs0 = avsp.tile([66, QC], F32, tag="avs0", name="avs0")
                avs1 = avsp.tile([66, QC], F32, tag="avs1", name="avs1")
                nc.vector.tensor_copy(out=avs0, in_=self.av0)
                nc.vector.tensor_copy(out=avs1, in_=self.av1)
                return avs0, avs1

        # ---- prologue: minimum pair-0 pieces inline (k0, q0 only;
        # v0/vtok0 ride in the filler ahead of av kt0's need) ----
        for g in (_qkv_chunk(0, 0, 0), _qkv_chunk(0, 2, 0)):
            for _ in g:
                pass

        # ---- flat software-pipelined sweep stream (lag-2 av) ----
        f = Filler()
        prev = None
        for idx in range(NPAIR * NSW):
            p, sw = divmod(idx, NSW)
            if sw == 0:
                if p == 0:
                    f.push_back(head_jit(0, skip0=True))
                    f.push_back(phase_a_tail(0))
                    f.push_back(head_jit(1))
                elif p < NPAIR - 1:
                    f.push_back(phase_a_tail(p))
                    f.push_back(head_jit(p + 1))
                else:
                    f.push_back(phase_a_tail(p))
            cur = SweepState(p, sw)
            if idx == 0:
                pkt = 900
            elif p == 0:
                pkt = 600
            elif p == NPAIR - 1:
                pkt = (550, 600, 650, 650)[sw]
            else:
                pkt = 430

            def ensure(pp, kind, n):
                while ready[(pp, kind)] < n:
                    assert f.q, f"filler underrun: {kind}{n} pair {pp}"
                    f.pump(213)

            for kt in range(KT):
                ensure(p, "q", sw + 1)
                ensure(p, "k", kt // 4 + 1)
                cur.scores(kt)
                if kt >= 2:
                    ensure(p, "vt", (kt - 2) // 4 + 1)
                    cur.av(kt - 2)
                elif prev is not None:
                    prev.av(KT - 2 + kt)
                    if kt == 1:
                        avs0, avs1 = prev.drain()
                        ng = norm_gen(prev.p, prev.sw, avs0, avs1)
                        if p == NPAIR - 1:
                            # FIFO after any in-flight outproj: a bc psum
                            # allocation interleaved into an outproj si
                            # (which holds both psA bufs) deadlocks.
                            f.push_back(ng)
                            if sw > 0:
                                f.push_back(outproj(sw - 1))
                        else:
                            f.push_front(ng)
                f.pump(pkt)
            prev = cur
        # tail: finish the last sweep, leftover fillers (must complete
        # before the inline norm — bc vs in-flight outproj psA bufs),
        # final norm + last outproj
        prev.av(KT - 2)
        prev.av(KT - 1)
        avs0, avs1 = prev.drain()
        f.drain()
        for _ in norm_gen(prev.p, prev.sw, avs0, avs1):
            pass
        for _ in outproj(NSW - 1, tail=True):
            pass

    nc.compile()
    return nc


def make_in_maps(x, W_qkv, b_qkv, W_out):
    """Build per-core input dicts (core c = batch c//2, head-group c%2)."""
    xb = x.reshape(B, T, D).astype(ml_dtypes.bfloat16)
    xts = [
        np.ascontiguousarray(
            xb[b].T.reshape(8, 128, T).transpose(1, 0, 2).reshape(128, 8 * T)
        )
        for b in range(B)
    ]
    in_maps = []
    for c in range(NCORES):
        b, g = c // 2, c % 2
        wq_cols, bq_parts = [], []
        for p in range(NPAIR):
            h0 = g * 8 + 2 * p
            lo, hi = h0 * Dh, (h0 + 2) * Dh  # two heads' 128 dims
            for sec in (1, 2, 0):  # k, v, q sections of W_qkv
                wq_cols.append(W_qkv[:, sec * D + lo : sec * D + hi])
                bq_parts.append(b_qkv[sec * D + lo : sec * D + hi])
        wq = np.concatenate(wq_cols, axis=1)  # [1024, 1536]
        wq = np.ascontiguousarray(
            wq.reshape(8, 128, 12, 128).transpose(1, 2, 0, 3).reshape(128, -1)
        ).astype(ml_dtypes.bfloat16)
        bq = np.ascontiguousarray(
            np.concatenate(bq_parts).reshape(12, 128).T
        ).astype(np.float32)
        wo = np.ascontiguousarray(
            W_out[g * 512 : (g + 1) * 512, :].reshape(4, 128, D)
            .transpose(1, 0, 2).reshape(128, -1)
        ).astype(ml_dtypes.bfloat16)
        in_maps.append(
            {
                "x": xts[b],
                "wqkv": wq,
                "bqkv": bq,
                "wout": wo,
            }
        )
    return in_maps


def kernel(x, W_qkv, b_qkv, W_out, b_out):
    x = np.asarray(x, dtype=np.float32)
    W_qkv = np.asarray(W_qkv, dtype=np.float32)
    b_qkv = np.asarray(b_qkv, dtype=np.float32)
    W_out = np.asarray(W_out, dtype=np.float32)
    b_out = np.asarray(b_out, dtype=np.float32)

    if "nc" not in _CACHE:
        _CACHE["nc"] = _build()
    nc = _CACHE["nc"]

    in_maps = make_in_maps(x, W_qkv, b_qkv, W_out)
    res = run_bass_kernel_spmd(nc, in_maps, core_ids=list(range(NCORES)))
    outp = np.empty((B, T, D), dtype=np.float32)
    for b in range(B):
        outp[b] = (
            res.results[2 * b]["out"].astype(np.float32)
            + res.results[2 * b + 1]["out"].astype(np.float32)
            + b_out
        )
    return outp


# revision 31
# speedup vs baseline: 1.0516x; 1.0516x over previous
"""Multi-head attention TRN2 kernel (nn_MultiHeadAttention_69922067579127).

Full-input contract: kernel(**inputs) takes the complete tensors and
returns the complete output. Sharding: batch x head-group hybrid —
core c = (batch c//2, head-group c%2) handles one batch (2048 tokens)
and 8 heads. The host pre-permutes all per-core operands (x^T, QKV/out
weight tiles in bf16) and sums the two per-group partial output
projections per batch, adding b_out once.

All matmuls run in bf16 with fp32 PSUM accumulation; rel err ~7e-3
against the 2e-2 gate.

The kernel is a two-engine balance: PE true work ~283us (QKV 82,
scores 55 concurrent-pair, AV 109, outproj 27, misc ~10) vs ACT exp
~285us (256 x [128,1024] EXP at 1114ns). The schedule keeps both
saturated:

  - scores: two K=64 head matmuls on row groups 0/64 run CONCURRENTLY
    on the PE (hw row-group tiling) — one 512-row slot per kt.
  - the 16 sweeps form ONE flat software-pipelined stream: scores run
    2 kt ahead of av, ACROSS sweep boundaries, so the ACT exp stream
    never sees a boundary bubble. The previous sweep's last two avs,
    its psum->SBUF drains (fp32), and its deferred normalization all
    interleave into the next sweep's first kts. Normalization uses no
    PE: denominator row -> partition 0 (small DMA), DVE
    approx-reciprocal, GPSIMD partition_broadcast across 64
    partitions, DVE multiplies; h1 rows reach partitions 64:128 via
    SBUF DMA.
  - per-kt PE slack (~475ns under the 1114ns ACT pace) is filled from
    a priority deque of filler generators (JIT-ordered next-pair QKV,
    vtok transposes, deferred norms, output projections), pumped by
    approximate PE-cost accounting. Pair-p QKV spills naturally into
    pair p's own first sweep (deadline: k chunk c by kt 4c).
  - prologue: DMA ordered so all of k0/q0/v0 weights + x chunk0 land
    ASAP on both queues; only k/q/v chunk0 + vtok0 of pair 0 run
    before the first sweep.

Emission-order safety: Tile derives dependencies from program order,
so a consumer emitted before its producer reads stale data silently.
The `ready` counters + ensure() force-pump the filler until a needed
k/q/vtok chunk's instructions are emitted before any scores/av that
reads them (this is what makes the JIT spillover of a pair's QKV into
its own first sweep safe).

Engine budget per core: PE ~306us true work (scores pair runs at
~320ns — row-tiled concurrency is ~1.5x, not 2x), ACT exp 287us, DVE
~160us. Measured HW exec ~390us on a cool device (vs 410us for the
previous schedule, 752us original baseline); hot-device (DVFS) runs
of identical code measure ~470us. Remaining gap to the ~330us floor:
~20us DMA/NEFF init prologue, ~16us pair-3/outproj tail, per-kt
ldweights/transition friction (~120ns/kt).
"""

import sys

sys.path.insert(0, "/opt/trn_rl_repo")

from collections import deque
from contextlib import ExitStack

import ml_dtypes
import numpy as np

import concourse.bacc as bacc
import concourse.mybir as mybir
import concourse.tile as tile
from concourse.bass_utils import run_bass_kernel_spmd
from concourse.masks import make_identity

F32 = mybir.dt.float32
BF16 = mybir.dt.bfloat16
EXP = mybir.ActivationFunctionType.Exp

B, T, D = 4, 2048, 1024
H, Dh = 16, 64
NCORES = 8
NPAIR = 4             # head-pairs per core (8 heads)
TC = 512              # token chunk for QKV
NTC = T // TC         # 4
KT = T // 128         # 16 key tiles
QC = 512              # queries per sweep
NSW = T // QC         # 4 sweeps

_CACHE = {}


class Filler:
    """Priority deque of filler generators yielding approx PE-ns costs."""

    def __init__(self):
        self.q = deque()
        self.debt = 0

    def push_front(self, gen):
        self.q.appendleft(gen)

    def push_back(self, gen):
        self.q.append(gen)

    def pump(self, ns):
        # debt-carrying: average consumption matches the budget even
        # though generator yield costs are coarse (a 426ns QKV yield
        # against a 450ns budget must not consume two yields).
        ns -= self.debt
        self.debt = 0
        while ns > 0 and self.q:
            try:
                c = next(self.q[0])
            except StopIteration:
                self.q.popleft()
                continue
            ns -= max(c if c else 0, 40)
        if ns < 0:
            self.debt = -ns

    def drain(self):
        while self.q:
            for _ in self.q.popleft():
                pass


def _build():
    nc = bacc.Bacc("TRN2", target_bir_lowering=False, debug=False)
    # host pre-transposed x^T: [ki, ko*T]
    x = nc.dram_tensor("x", [128, 8 * T], BF16, kind="ExternalInput").ap()
    # host pre-permuted: wqkv [ki, m*ko*n], bqkv [p, m], wout [p, m*n]
    wqkv = nc.dram_tensor("wqkv", [128, 12 * 8 * 128], BF16, kind="ExternalInput").ap()
    bqkv = nc.dram_tensor("bqkv", [128, 12], F32, kind="ExternalInput").ap()
    wout = nc.dram_tensor("wout", [128, 4 * D], BF16, kind="ExternalInput").ap()
    out = nc.dram_tensor("out", [T, D], BF16, kind="ExternalOutput").ap()

    with tile.TileContext(nc) as tc, ExitStack() as ctx:
        const = ctx.enter_context(tc.tile_pool(name="const", bufs=1))
        big = ctx.enter_context(tc.tile_pool(name="big", bufs=1))
        stp = ctx.enter_context(tc.tile_pool(name="stp", bufs=4))
        work = ctx.enter_context(tc.tile_pool(name="work", bufs=2))
        avsp = ctx.enter_context(tc.tile_pool(name="avsp", bufs=2))
        outp = ctx.enter_context(tc.tile_pool(name="outp", bufs=3))
        # PSUM: 8 banks. sc 2x2 + av 2x1 + mm 2x1 = 8.
        pssc = ctx.enter_context(tc.tile_pool(name="pssc", bufs=2, space="PSUM"))
        psav = ctx.enter_context(tc.tile_pool(name="psav", bufs=2, space="PSUM"))
        psA = ctx.enter_context(tc.tile_pool(name="psA", bufs=2, space="PSUM"))

        # ---- constants ----
        ones_b = const.tile([128, 64], BF16)
        nc.vector.memset(ones_b, 1.0)
        ident = const.tile([128, 128], BF16)
        make_identity(nc, ident)

        # ---- persistent per-core tiles ----
        xt = big.tile([128, 8, T], BF16)       # x^T
        qT = big.tile([128, NPAIR, T], BF16)   # per pair: [h_even|h_odd] dims
        kT = big.tile([128, NPAIR, T], BF16)
        vTt = big.tile([128, NPAIR, T], BF16)  # V^T staging (transpose source)
        # token-major V: per (kt, pair): [v_h0(64) | 1 1 | v_h1(64) | 1 1]
        vtok = big.tile([128, KT, NPAIR, 132], BF16)
        attnT = big.tile([128, NPAIR, T], BF16)

        vtok5 = vtok.rearrange("a k p (h c) -> a k p h c", c=66)
        nc.gpsimd.memset(vtok5[:, :, :, :, 64:66], 1.0)

        w_r = const.tile([128, 12, 8, 128], BF16)
        wq_v = wqkv.rearrange("a (m f) -> a m f", m=12)
        w_rv = w_r.rearrange("a m ko n -> a m (ko n)")
        bq_sb = const.tile([128, 12], F32)
        wo_r = const.tile([128, 4, D], BF16)
        x_v = x.rearrange("a (ko t) -> a ko t", ko=8)

        # ---- DMA prologue: JIT ordering, both queues balanced.
        # Queue BW ~135GB/s each; everything ordered by first-use
        # deadline (x chunk c by scores kt 4c, w m-tile by its QKV).
        def _x(q, tci, half):
            ko = slice(0, 4) if half == 0 else slice(4, 8)
            q.dma_start(
                out=xt[:, ko, tci * TC : (tci + 1) * TC],
                in_=x_v[:, ko, tci * TC : (tci + 1) * TC],
            )

        def _w(q, m):
            q.dma_start(out=w_rv[:, m : m + 1], in_=wq_v[:, m : m + 1])

        def _x2(q, tci, kolo, kohi):
            q.dma_start(
                out=xt[:, kolo:kohi, tci * TC : (tci + 1) * TC],
                in_=x_v[:, kolo:kohi, tci * TC : (tci + 1) * TC],
            )

        _x2(nc.sync, 0, 0, 2)
        _w(nc.scalar, 0)                     # k0 w
        nc.scalar.dma_start(out=bq_sb, in_=bqkv)
        _x2(nc.sync, 0, 2, 4)
        _x2(nc.scalar, 0, 4, 6)
        _x2(nc.scalar, 0, 6, 8)
        _w(nc.sync, 2)                       # q0 w
        _w(nc.scalar, 1)                     # v0 w
        _x(nc.sync, 1, 0)
        _x(nc.scalar, 1, 1)
        _x(nc.sync, 2, 0)
        _x(nc.scalar, 2, 1)
        _w(nc.sync, 3)                       # pair1 k w
        _w(nc.sync, 5)                       # pair1 q w
        _x(nc.sync, 3, 0)
        _x(nc.scalar, 3, 1)
        _w(nc.scalar, 4)                     # pair1 v w
        _w(nc.sync, 6)
        nc.sync.dma_start(out=wo_r, in_=wout.rearrange("a (m n) -> a m n", m=4))
        _w(nc.scalar, 8)
        _w(nc.scalar, 7)
        _w(nc.sync, 9)
        _w(nc.scalar, 11)
        _w(nc.sync, 10)

        # emission-order readiness: ready[(p, kind)] counts chunks whose
        # instructions are fully EMITTED; consumers must ensure() their
        # producer is emitted first or Tile sees read-before-write.
        ready = {}
        for _p in range(NPAIR):
            for _kind in ("k", "v", "q", "vt"):
                ready[(_p, _kind)] = 0

        def _qkv_chunk(p, mi, tci):
            """Generator: one QKV m-tile for one 512-token chunk."""
            m = 3 * p + mi
            t0 = tci * TC
            psq = psA.tile([128, TC], F32, tag="mm", name="psq")
            for ko in range(8):
                nc.tensor.matmul(
                    psq,
                    w_r[:, m, ko, :],
                    xt[:, ko, t0 : t0 + TC],
                    start=(ko == 0),
                    stop=(ko == 7),
                )
                if ko < 7:
                    yield 213
            dst = (kT, vTt, qT)[mi]
            nc.vector.tensor_scalar_add(
                out=dst[:, p, t0 : t0 + TC],
                in0=psq,
                scalar1=bq_sb[:, m : m + 1],
            )
            ready[(p, ("k", "v", "q")[mi])] += 1
            yield 213

        def _vtok_chunk(p, tci):
            """Generator: token-major V for one chunk's 4 key-tiles."""
            pst = psA.tile([128, TC], BF16, tag="mm", name="pst")
            for j in range(4):
                kt0 = tci * 4
                nc.tensor.transpose(
                    pst[:, j * 128 : (j + 1) * 128],
                    vTt[:, p, (kt0 + j) * 128 : (kt0 + j + 1) * 128],
                    ident,
                )
                yield 53
            nc.vector.tensor_copy(
                out=vtok5[:, tci * 4 : tci * 4 + 4, p, :, 0:64],
                in_=pst.rearrange("a (j h c) -> a j h c", j=4, c=64),
            )
            ready[(p, "vt")] += 1
            yield 40

        def head_jit(p, skip0=False):
            """Pair-p phase A in just-in-time order: k/q/v chunk 0 first,
            then per chunk c: k(c), v(c), vtok(c); q chunks 1-3 last
            (deadline: sweep sw needs q chunk sw)."""
            if not skip0:
                yield from _qkv_chunk(p, 0, 0)
                yield from _qkv_chunk(p, 2, 0)
            yield from _qkv_chunk(p, 1, 0)
            yield from _vtok_chunk(p, 0)
            for tci in range(1, NTC):
                yield from _qkv_chunk(p, 0, tci)
                yield from _qkv_chunk(p, 1, tci)
                yield from _vtok_chunk(p, tci)

        def phase_a_tail(p):
            """q chunks 1-3: chunk sw only needed by sweep sw."""
            for tci in range(1, NTC):
                yield from _qkv_chunk(p, 2, tci)

        def outproj(sw, tail=False):
            """Generator: output projection for sweep sw's 512 tokens,
            psum-accumulated over all 4 pairs."""
            for si in range(4):
                sl = sw * 4 + si
                outsb = outp.tile([128, D], BF16, tag="outsb", name="outsb")
                pos = [
                    psA.tile([128, QC], F32, tag="mm", name=f"po{n2}")
                    for n2 in range(2)
                ]
                for p in range(NPAIR):
                    for n2 in range(2):
                        nc.tensor.matmul(
                            pos[n2],
                            attnT[:, p, sl * 128 : (sl + 1) * 128],
                            wo_r[:, p, n2 * QC : (n2 + 1) * QC],
                            start=(p == 0),
                            stop=(p == NPAIR - 1),
                        )
                    yield 426
                nc.vector.tensor_copy(out=outsb[:, 0:QC], in_=pos[0])
                if tail:  # ACT is idle after the last exp
                    nc.scalar.activation(
                        out=outsb[:, QC : 2 * QC],
                        in_=pos[1],
                        func=mybir.ActivationFunctionType.Copy,
                    )
                else:  # during attention ACT paces the kt loop - use DVE
                    nc.vector.tensor_copy(
                        out=outsb[:, QC : 2 * QC], in_=pos[1]
                    )
                for n2 in range(2):
                    eng = nc.sync if (sl + n2) % 2 == 0 else nc.scalar
                    eng.dma_start(
                        out=out[sl * 128 : (sl + 1) * 128, n2 * QC : (n2 + 1) * QC],
                        in_=outsb[:, n2 * QC : (n2 + 1) * QC],
                    )
                yield 40

        def norm_gen(p, sw, avs0, avs1):
            """Deferred normalization of sweep (p, sw) from the SBUF
            drains: approx reciprocal on the denominator row -> DMA
            partition-broadcast (replaces the PE ones-matmul) ->
            multiply into attnT; h1 rows move to partitions 64:128."""
            q0 = sw * QC
            for h, avs in ((0, avs0), (1, avs1)):
                denrow = work.tile([1, QC], F32, tag=f"denrow{h}", name=f"denrow{h}")
                eng = nc.sync if h == 0 else nc.scalar
                eng.dma_start(out=denrow, in_=avs[64:65, :])
                yield 0
                recrow = work.tile([1, QC], F32, tag=f"recrow{h}", name=f"recrow{h}")
                nc.vector.reciprocal_approx_fast(out=recrow, in_=denrow)
                yield 40
                rec = work.tile([64, QC], F32, tag=f"rec{h}", name=f"rec{h}")
                nc.gpsimd.partition_broadcast(rec, recrow)
                yield 0
                if h == 0:
                    nc.vector.tensor_mul(
                        out=attnT[0:64, p, q0 : q0 + QC], in0=avs[0:64, :], in1=rec
                    )
                    yield 40
                else:
                    tmp1 = work.tile([64, QC], BF16, tag="tmp1", name="tmp1")
                    nc.vector.tensor_mul(out=tmp1, in0=avs[0:64, :], in1=rec)
                    yield 40
                    nc.sync.dma_start(
                        out=attnT[64:128, p, q0 : q0 + QC], in_=tmp1
                    )
                    yield 0

        class SweepState:
            """One 512-query sweep's score/av emission state."""

            def __init__(self, p, sw):
                self.p, self.sw = p, sw
                self.q0 = sw * QC
                self.sts = [None] * KT
                self.av0 = self.av1 = None

            def scores(self, kt):
                p, q0 = self.p, self.q0
                sc = pssc.tile([128, 1024], F32, tag="sc", name="sc")
                nc.tensor.matmul(
                    sc[:, 0:QC],
                    kT[0:64, p, kt * 128 : (kt + 1) * 128],
                    qT[0:64, p, q0 : q0 + QC],
                    start=True,
                    stop=True,
                )
                nc.tensor.matmul(
                    sc[:, QC : 2 * QC],
                    kT[64:128, p, kt * 128 : (kt + 1) * 128],
                    qT[64:128, p, q0 : q0 + QC],
                    start=True,
                    stop=True,
                )
                st = stp.tile([128, 1024], BF16, tag="st", name="st")
                nc.scalar.activation(out=st, in_=sc, func=EXP, scale=0.125)
                self.sts[kt] = st

            def av(self, kt):
                p = self.p
                if self.av0 is None:
                    self.av0 = psav.tile([66, QC], F32, tag="av", name="av0")
                    self.av1 = psav.tile([66, QC], F32, tag="av", name="av1")
                st = self.sts[kt]
                nc.tensor.matmul(
                    self.av0,
                    vtok[:, kt, p, 0:66],
                    st[:, 0:QC],
                    start=(kt == 0),
                    stop=(kt == KT - 1),
                )
                nc.tensor.matmul(
                    self.av1,
                    vtok[:, kt, p, 66:132],
                    st[:, QC : 2 * QC],
                    start=(kt == 0),
                    stop=(kt == KT - 1),
                )
                self.sts[kt] = None

            def drain(self):
                avs0 = avsp.tile([66, QC], F32, tag="avs0", name="avs0")
                avs1 = avsp.tile([66, QC], F32, tag="avs1", name="avs1")
                nc.vector.tensor_copy(out=avs0, in_=self.av0)
                nc.vector.tensor_copy(out=avs1, in_=self.av1)
                return avs0, avs1

        # ---- prologue: minimum pair-0 pieces inline (k0, q0 only;
        # v0/vtok0 ride in the filler ahead of av kt0's need) ----
        for g in (_qkv_chunk(0, 0, 0), _qkv_chunk(0, 2, 0)):
            for _ in g:
                pass

        # ---- flat software-pipelined sweep stream (lag-2 av) ----
        f = Filler()
        prev = None
        for idx in range(NPAIR * NSW):
            p, sw = divmod(idx, NSW)
            if sw == 0:
                if p == 0:
                    f.push_back(head_jit(0, skip0=True))
                    f.push_back(phase_a_tail(0))
                    f.push_back(head_jit(1))
                elif p < NPAIR - 1:
                    f.push_back(phase_a_tail(p))
                    f.push_back(head_jit(p + 1))
                else:
                    f.push_back(phase_a_tail(p))
            cur = SweepState(p, sw)
            if idx == 0:
                pkt = 900
            elif p == 0:
                pkt = 600
            elif p == NPAIR - 1:
                pkt = (550, 600, 650, 650)[sw]
            else:
                pkt = 430

            def ensure(pp, kind, n):
                while ready[(pp, kind)] < n:
                    assert f.q, f"filler underrun: {kind}{n} pair {pp}"
                    f.pump(213)

            # 2-kt steps: burst both scores pairs, then all four avs,
            # then filler — halves the filler->scores and scores->av
            # matmul-type transitions (~100ns each of PE friction).
            for kt2 in range(0, KT, 2):
                ensure(p, "q", sw + 1)
                ensure(p, "k", (kt2 + 1) // 4 + 1)
                cur.scores(kt2)
                cur.scores(kt2 + 1)
                if kt2 >= 2:
                    ensure(p, "vt", (kt2 - 1) // 4 + 1)
                    cur.av(kt2 - 2)
                    cur.av(kt2 - 1)
                elif prev is not None:
                    prev.av(KT - 2)
                    prev.av(KT - 1)
                    avs0, avs1 = prev.drain()
                    ng = norm_gen(prev.p, prev.sw, avs0, avs1)
                    if p == NPAIR - 1:
                        # FIFO after any in-flight outproj: a psum
                        # allocation interleaved into an outproj si
                        # (which holds both psA bufs) deadlocks.
                        f.push_back(ng)
                        if sw > 0:
                            f.push_back(outproj(sw - 1))
                    else:
                        f.push_front(ng)
                f.pump(2 * pkt)
            prev = cur
        # tail: finish the last sweep, leftover fillers (must complete
        # before the inline norm — bc vs in-flight outproj psA bufs),
        # final norm + last outproj
        prev.av(KT - 2)
        prev.av(KT - 1)
        avs0, avs1 = prev.drain()
        f.drain()
        for _ in norm_gen(prev.p, prev.sw, avs0, avs1):
            pass
        for _ in outproj(NSW - 1, tail=True):
            pass

    nc.compile()
    return nc


def make_in_maps(x, W_qkv, b_qkv, W_out):
    """Build per-core input dicts (core c = batch c//2, head-group c%2)."""
    xb = x.reshape(B, T, D).astype(ml_dtypes.bfloat16)
    xts = [
        np.ascontiguousarray(
            xb[b].T.reshape(8, 128, T).transpose(1, 0, 2).reshape(128, 8 * T)
        )
        for b in range(B)
    ]
    in_maps = []
    for c in range(NCORES):
        b, g = c // 2, c % 2
        wq_cols, bq_parts = [], []
        for p in range(NPAIR):
            h0 = g * 8 + 2 * p
            lo, hi = h0 * Dh, (h0 + 2) * Dh  # two heads' 128 dims
            for sec in (1, 2, 0):  # k, v, q sections of W_qkv
                wq_cols.append(W_qkv[:, sec * D + lo : sec * D + hi])
                bq_parts.append(b_qkv[sec * D + lo : sec * D + hi])
        wq = np.concatenate(wq_cols, axis=1)  # [1024, 1536]
        wq = np.ascontiguousarray(
            wq.reshape(8, 128, 12, 128).transpose(1, 2, 0, 3).reshape(128, -1)
        ).astype(ml_dtypes.bfloat16)
        bq = np.ascontiguousarray(
            np.concatenate(bq_parts).reshape(12, 128).T
        ).astype(np.float32)
        wo = np.ascontiguousarray(
            W_out[g * 512 : (g + 1) * 512, :].reshape(4, 128, D)
            .transpose(1, 0, 2).reshape(128, -1)
        ).astype(ml_dtypes.bfloat16)
        in_maps.append(
            {
                "x": xts[b],
                "wqkv": wq,
                "bqkv": bq,
                "wout": wo,
            }
        )
    return in_maps


def kernel(x, W_qkv, b_qkv, W_out, b_out):
    x = np.asarray(x, dtype=np.float32)
    W_qkv = np.asarray(W_qkv, dtype=np.float32)
    b_qkv = np.asarray(b_qkv, dtype=np.float32)
    W_out = np.asarray(W_out, dtype=np.float32)
    b_out = np.asarray(b_out, dtype=np.float32)

    if "nc" not in _CACHE:
        _CACHE["nc"] = _build()
    nc = _CACHE["nc"]

    in_maps = make_in_maps(x, W_qkv, b_qkv, W_out)
    res = run_bass_kernel_spmd(nc, in_maps, core_ids=list(range(NCORES)))
    outp = np.empty((B, T, D), dtype=np.float32)
    for b in range(B):
        outp[b] = (
            res.results[2 * b]["out"].astype(np.float32)
            + res.results[2 * b + 1]["out"].astype(np.float32)
            + b_out
        )
    return outp
